# revision 19
# baseline (speedup 1.0000x reference)
"""Trainium2 Bass kernel for BinarizeConv2dSDP.

Math (reference):
    s   = M + rv @ Z          (the rsqrt normalization is sign-preserving:
                               w = (m + rv@z) * rsqrt(...) with rsqrt > 0,
                               so sign(w) == sign(s))
    bw  = sign(s)             (O, I, 3, 3)
    ba  = sign(x)             (B, C, H, W)
    out = conv2d(ba, bw, stride 1, pad 1) * Alpha

Strategy:
    - Data-parallel over batch: 8 cores x 4 images each.
    - M/Z are passed host-pretransposed into the matmul lhsT layout so
      weight synthesis is pure elementwise: 5 fused scalar_tensor_tensor
      rounds + sign, chunked into 4 tap-group tiles (3 on DVE, 1 on
      GpSimd) that pipeline behind the halved per-Z DMA arrivals.
    - Each image is signed into TWO fp8 padded buffers: A (normal) and
      B (shifted left one pixel), written by one ACT op per row-chunk via
      a broadcast (stride-0) input dim.  A vertical tap pair steps 64B
      inside A; a horizontal tap pair steps A->B (3712B).  Both are legal
      DoubleRow strides, so each conv row-tile is 4 DoubleRow matmuls
      (K=256) + 1 normal fp8 matmul instead of 9 taps:
        horiz pairs (ky, kx0|kx1) x3  +  vert pair (ky0|ky1, kx2)  +
        single (ky2, kx2)
      +-1 is exact in fp8 and PSUM accumulates f32, so results are exact.
    - Wire order (one HWDGE queue): alpha, x0h0, M, Z0..Z4 (halved),
      x0h1, x1..x3 (halved).  Output stores ride the gpsimd SWDGE queue.
    - Alpha applied during PSUM->SBUF evacuation on DVE; f32 out.
"""

import os
import numpy as np

import concourse.bass as bass
import concourse.tile as tile
from concourse import bacc, mybir
from concourse.bass_utils import run_bass_kernel_spmd

F32 = mybir.dt.float32
BF16 = mybir.dt.bfloat16
FP8 = mybir.dt.float8e4

USE_FP8 = bool(int(os.environ.get("BASS_KERNEL_FP8", "1")))

B_FULL = 32
N_CORES = 8
B_CORE = B_FULL // N_CORES  # 4 images per core
C = 128      # in channels
O = 128      # out channels
H = W = 56
HH = H // 2
HP = 58                      # padded rows
WP = 64 if USE_FP8 else 58   # padded row stride (64 -> vert tap pair = 64B)
IMG = HP * WP                # one padded image copy (3712 fp8), 16-aligned
KS = 3
NTAPS = KS * KS
IKK = C * NTAPS  # 1152
ROWS_PER_TILE = 8           # output rows per PSUM tile -> N = 8*56 = 448
N_TILE = ROWS_PER_TILE * W  # 448 fp32 <= 512 (one PSUM bank)
N_ROW_TILES = H // ROWS_PER_TILE  # 7
ADT = FP8 if USE_FP8 else BF16
SIGN_CHUNK = 14             # x rows per dual-sign chunk (4 chunks)

# lhsT layout inside the packed [C, 1152] weight vector:
#   horiz pairs: ky*256 + j*128 + o   (j = kx in {0,1})          (768)
#   vert pair  : 768 + j*128 + o      (j = ky in {0,1}, kx = 2)  (256)
#   single     : 1024 + o             (ky = 2, kx = 2)           (128)
# Weight-synthesis chunk tiles: c0/c1/c2 = horiz ky row (256 each),
# c3 = vert pair + single (384).  DMA halves split at column 512.
CH_SZ = [256, 256, 256, 384]
ZH = 512  # z/m half split (covers c0+c1; second half covers c2+c3)


def pack_zm(w):
    """(O, C, 3, 3) f32 -> (C, 1152) in lhsT tap order."""
    hp = np.transpose(w[:, :, :, 0:2], (1, 2, 3, 0))      # (C, ky, kx2, O)
    vp = np.transpose(w[:, :, 0:2, 2], (1, 2, 0))         # (C, ky2, O)
    sg = np.transpose(w[:, :, 2, 2], (1, 0))              # (C, O)
    return np.concatenate(
        [hp.reshape(C, -1), vp.reshape(C, -1), sg.reshape(C, -1)], axis=1
    )


def build_program(rv: np.ndarray, n_img: int = B_CORE):
    """Build the per-core Bass program. rv values are baked as immediates."""
    nc = bacc.Bacc(
        "TRN2",
        target_bir_lowering=False,
        debug=False,
        num_devices=N_CORES,
    )

    x_t = nc.dram_tensor("x", (n_img, C, H, W), F32, kind="ExternalInput").ap()
    a_t = nc.dram_tensor("Alpha", (O, 1, 1), F32, kind="ExternalInput").ap()
    # ZM[k] for k<5 is Z_k, ZM[5] is M, all in lhsT tap order.
    zm_t = nc.dram_tensor("ZM", (6, C, IKK), F32, kind="ExternalInput").ap()
    out_t = nc.dram_tensor("out", (n_img, O, H, W), F32, kind="ExternalOutput").ap()

    rv = np.asarray(rv, dtype=np.float32).reshape(-1)
    assert rv.shape[0] == 5

    with tile.TileContext(nc) as tc:
        with (
            tc.tile_pool(name="const", bufs=1) as const_pool,
            tc.tile_pool(name="wsyn", bufs=1) as wsyn_pool,
            tc.tile_pool(name="imgs", bufs=1) as img_pool,
            tc.tile_pool(name="xstage", bufs=8) as x_pool,
            tc.tile_pool(name="evac", bufs=8) as ev_pool,
            tc.tile_pool(name="cpsum", bufs=7, space="PSUM") as cpsum_pool,
        ):
            # --- wire order: alpha, x0h0, M, Z0..Z4 (halved to amortize the
            # ~2.5us completion-receipt lag), x0h1, x1..x3 halves.
            # Halves/chunks are separate tiles so dependency tracking stays
            # at half granularity.
            alpha_sb = const_pool.tile([O, 1], F32)
            nc.sync.dma_start(alpha_sb, a_t.rearrange("o a b -> o (a b)"))

            x_tiles = [None] * n_img  # per image: (half0, half1) tiles

            def alloc_x(img):
                x_tiles[img] = tuple(
                    x_pool.tile([C, HH * W], F32, name=f"x{img}h{hb}", tag="xin")
                    for hb in range(2)
                )

            def load_x_half(img, hb):
                nc.sync.dma_start(
                    x_tiles[img][hb].rearrange("c (h w) -> c h w", w=W),
                    x_t[img, :, hb * HH : (hb + 1) * HH, :],
                )

            def load_halved(dsts, src):
                nc.sync.dma_start(dsts[0], src[:, 0:ZH])
                nc.sync.dma_start(dsts[1], src[:, ZH:IKK])

            def wtiles(name):
                return (
                    wsyn_pool.tile([C, ZH], F32, name=f"{name}h0"),
                    wsyn_pool.tile([C, IKK - ZH], F32, name=f"{name}h1"),
                )

            alloc_x(0)
            load_x_half(0, 0)
            load_x_half(0, 1)
            m_h = wtiles("m")
            load_halved(m_h, zm_t[5])
            z_hs = []
            for k in range(5):
                z_h = wtiles(f"z{k}")
                load_halved(z_h, zm_t[k])
                z_hs.append(z_h)
            for img in range(1, n_img):
                alloc_x(img)
                load_x_half(img, 0)
                load_x_half(img, 1)

            # --- per-image padded sign(x) buffers.  A = sign(x) with zeroed
            # borders; B = A shifted one byte left (so B[h,w] = A[h,w+1]),
            # produced by a gpsimd SBUF->SBUF DMA — B's borders come along
            # for free.  A and B live in one tile so all writes order
            # before the conv reads.
            padded = []
            for img in range(n_img):
                pd = img_pool.tile(
                    [C, 2 * IMG], ADT, name=f"pad{img}", tag=f"pad{img}"
                )
                pdA = pd[:, 0:IMG].rearrange("p (h w) -> p h w", w=WP)
                pdB = pd[:, IMG : 2 * IMG].rearrange("p (h w) -> p h w", w=WP)
                nc.gpsimd.memset(pdA[:, 0, 0:HP], 0.0)
                nc.gpsimd.memset(pdA[:, HP - 1, 0:HP], 0.0)
                nc.gpsimd.memset(pdA[:, 1 : HP - 1, 0:1], 0.0)
                nc.gpsimd.memset(pdA[:, 1 : HP - 1, HP - 1 : HP], 0.0)
                if WP > HP:
                    # row-slack bytes flow through the shifted B copy; zero
                    # them so no garbage fp8 (possibly NaN) is ever read
                    nc.gpsimd.memset(pdA[:, 0:HP, HP:WP], 0.0)
                padded.append((pd, pdA, pdB))

            def sign_half(img, hb):
                pd, pdA, pdB = padded[img]
                r0 = hb * HH
                src = x_tiles[img][hb].rearrange("c (h w) -> c h w", w=W)
                nc.scalar.sign(pdA[:, 1 + r0 : 1 + r0 + HH, 1 : 1 + W], src)

            def b_copy(img):
                """One shifted SBUF->SBUF copy A -> B on the scalar HWDGE
                ring (stores ride gpsimd's, so no head-of-line there)."""
                pd, _, _ = padded[img]
                nc.scalar.dma_start(pd[:, IMG : 2 * IMG - 1], pd[:, 1:IMG])

            sign_half(0, 0)
            sign_half(0, 1)
            b_copy(0)

            # --- weight synthesis: s = M + sum_k rv_k Z_k, already in lhsT
            # layout.  k outer so each round only needs Z_k; chunks c0..c2
            # on DVE, c3 on GpSimd; signs emitted in tile order so the conv
            # (which consumes c0 first) can start on partial weights.
            s_c = [
                wsyn_pool.tile([C, CH_SZ[c]], F32, name=f"s{c}") for c in range(4)
            ]
            bw_c = [
                wsyn_pool.tile([C, CH_SZ[c]], ADT, name=f"bw{c}") for c in range(4)
            ]

            def wslice(halves, c):
                if c == 0:
                    return halves[0][:, 0:256]
                if c == 1:
                    return halves[0][:, 256:512]
                if c == 2:
                    return halves[1][:, 0:256]
                return halves[1][:, 256:640]

            for k in range(5):
                for c in range(4):
                    nc.vector.scalar_tensor_tensor(
                        out=s_c[c],
                        in0=wslice(z_hs[k], c),
                        scalar=float(rv[k]),
                        in1=wslice(m_h, c) if k == 0 else s_c[c],
                        op0=mybir.AluOpType.mult,
                        op1=mybir.AluOpType.add,
                    )
                    if k == 4:
                        nc.scalar.sign(bw_c[c], s_c[c])

            def pair_ap(ct, off):
                return bw_c[ct][:, off : off + 2 * O].rearrange(
                    "c (j o) -> c j o", o=O
                )

            # --- main conv loop; the next image's sign halves are emitted
            # ahead (h0 at loop start, h1 + B copy after tile 2) so its
            # pad buffer completes before its own tiles need it.
            for img in range(n_img):
                if img + 1 < n_img:
                    sign_half(img + 1, 0)
                _, pdA, pdB = padded[img]

                for nt in range(N_ROW_TILES):
                    if nt == 3 and img + 1 < n_img:
                        sign_half(img + 1, 1)
                        b_copy(img + 1)
                    y0 = nt * ROWS_PER_TILE
                    cv = cpsum_pool.tile([O, N_TILE], F32, tag="cv")
                    if USE_FP8:
                        # horiz pairs (ky, kx0|kx1): step A->B (IMG bytes);
                        # needs bw chunk ky only -> earliest weights first
                        for ky in range(KS):
                            winh = pdA[:, y0 + ky : y0 + ky + ROWS_PER_TILE, 0:W]
                            aph = bass.AP(
                                winh.tensor,
                                winh.offset,
                                [list(winh.ap[0]), [IMG, 2]]
                                + [list(p) for p in winh.ap[1:]],
                            )
                            nc.tensor.matmul(
                                cv,
                                pair_ap(ky, 0),
                                aph,
                                start=(ky == 0),
                                stop=False,
                                perf_mode=mybir.MatmulPerfMode.DoubleRow,
                            )
                        # vert pair (ky0|ky1, kx=2): step WP inside A
                        winv = pdA[:, y0 : y0 + ROWS_PER_TILE, 2 : 2 + W]
                        apv = bass.AP(
                            winv.tensor,
                            winv.offset,
                            [list(winv.ap[0]), [WP, 2]]
                            + [list(p) for p in winv.ap[1:]],
                        )
                        nc.tensor.matmul(
                            cv,
                            pair_ap(3, 0),
                            apv,
                            start=False,
                            stop=False,
                            perf_mode=mybir.MatmulPerfMode.DoubleRow,
                        )
                        # single (ky2, kx2) from A
                        nc.tensor.matmul(
                            cv,
                            bw_c[3][:, 256 : 256 + O],
                            pdA[:, y0 + 2 : y0 + 2 + ROWS_PER_TILE, 2 : 2 + W],
                            start=False,
                            stop=True,
                        )
                    else:
                        t = 0
                        for ky in range(KS):
                            for kx in range(KS):
                                win = pdA[
                                    :,
                                    y0 + ky : y0 + ky + ROWS_PER_TILE,
                                    kx : kx + W,
                                ]
                                if kx < 2:
                                    lhsT = bw_c[ky][:, kx * O : (kx + 1) * O]
                                elif ky < 2:
                                    lhsT = bw_c[3][:, ky * O : (ky + 1) * O]
                                else:
                                    lhsT = bw_c[3][:, 256 : 256 + O]
                                nc.tensor.matmul(
                                    cv,
                                    lhsT,
                                    win,
                                    start=(t == 0),
                                    stop=(t == NTAPS - 1),
                                )
                                t += 1
                    ev = ev_pool.tile([O, N_TILE], F32, tag="ev")
                    nc.vector.tensor_scalar_mul(ev, cv, alpha_sb[:, 0:1])
                    # stores all ride the gpsimd SWDGE queue: the scalar
                    # engine is busy signing images and the sync ring is
                    # busy with x loads
                    nc.gpsimd.dma_start(
                        out_t[img, :, y0 : y0 + ROWS_PER_TILE, :],
                        ev.rearrange("o (h w) -> o h w", w=W),
                    )

    nc.compile()
    return nc


def _ensure_ntff_hook():
    """Register the axon NTFF profiling hook if the image's antenv lacks it.

    Only used when BASS_KERNEL_TRACE=1 (dev profiling); best-effort.
    """
    import sys
    import types

    try:
        import antenv

        if hasattr(antenv, "axon_hooks"):
            return
        mod = types.ModuleType("antenv.axon_hooks")
        _hook = [None]
        mod.set_axon_ntff_profile_hook = lambda h: _hook.__setitem__(0, h)
        mod.get_axon_ntff_profile_hook = lambda: _hook[0]
        sys.modules["antenv.axon_hooks"] = mod
        antenv.axon_hooks = mod
        from trn_agent_boot.trn_boot import _ntff_profile_via_ctypes

        mod.set_axon_ntff_profile_hook(
            _ntff_profile_via_ctypes("/opt/axon/libaxon_pjrt.so")
        )
    except Exception as e:  # pragma: no cover - profiling is optional
        print(f"NTFF hook registration failed ({e}); tracing disabled")


def kernel(x, Alpha, M, Z, rv):
    x = np.ascontiguousarray(np.asarray(x, dtype=np.float32))
    Alpha = np.ascontiguousarray(np.asarray(Alpha, dtype=np.float32))
    M = np.asarray(M, dtype=np.float32)
    Z = np.asarray(Z, dtype=np.float32)
    rv = np.asarray(rv, dtype=np.float32)

    trace = bool(int(os.environ.get("BASS_KERNEL_TRACE", "0")))
    if trace:
        _ensure_ntff_hook()

    nc = build_program(rv)

    zm = np.stack(
        [pack_zm(Z.reshape(5, O, C, KS, KS)[k]) for k in range(5)]
        + [pack_zm(M.reshape(O, C, KS, KS))]
    ).astype(np.float32)
    zm = np.ascontiguousarray(zm)

    in_maps = []
    for c in range(N_CORES):
        in_maps.append(
            {
                "x": np.ascontiguousarray(x[c * B_CORE : (c + 1) * B_CORE]),
                "Alpha": Alpha,
                "ZM": zm,
            }
        )

    res = run_bass_kernel_spmd(
        nc,
        in_maps,
        core_ids=list(range(N_CORES)),
        trace=trace,
    )
    out = np.concatenate([res.results[c]["out"] for c in range(N_CORES)], axis=0)
    if trace:
        kernel.last_results = res
    return out


# revision 21
# speedup vs baseline: 1.2331x; 1.2331x over previous
"""Trainium2 Bass kernel for BinarizeConv2dSDP.

Math (reference):
    s   = M + rv @ Z          (the rsqrt normalization is sign-preserving:
                               w = (m + rv@z) * rsqrt(...) with rsqrt > 0,
                               so sign(w) == sign(s))
    bw  = sign(s)             (O, I, 3, 3)
    ba  = sign(x)             (B, C, H, W)
    out = conv2d(ba, bw, stride 1, pad 1) * Alpha

Strategy:
    - Data-parallel over batch: 8 cores x 4 images each.
    - M/Z are passed host-pretransposed into the matmul lhsT layout so
      weight synthesis is pure elementwise: 5 fused scalar_tensor_tensor
      rounds + sign, chunked into 4 tap-group tiles (3 on DVE, 1 on
      GpSimd) that pipeline behind the halved per-Z DMA arrivals.
    - Each image is signed into TWO fp8 padded buffers: A (normal) and
      B (shifted left one pixel), written by one ACT op per row-chunk via
      a broadcast (stride-0) input dim.  A vertical tap pair steps 64B
      inside A; a horizontal tap pair steps A->B (3712B).  Both are legal
      DoubleRow strides, so each conv row-tile is 4 DoubleRow matmuls
      (K=256) + 1 normal fp8 matmul instead of 9 taps:
        horiz pairs (ky, kx0|kx1) x3  +  vert pair (ky0|ky1, kx2)  +
        single (ky2, kx2)
      +-1 is exact in fp8 and PSUM accumulates f32, so results are exact.
    - Wire order (one HWDGE queue): alpha, x0h0, M, Z0..Z4 (halved),
      x0h1, x1..x3 (halved).  Output stores ride the gpsimd SWDGE queue.
    - Alpha applied during PSUM->SBUF evacuation on DVE; f32 out.
"""

import os
import numpy as np

import concourse.bass as bass
import concourse.tile as tile
from concourse import bacc, mybir
from concourse.bass_utils import run_bass_kernel_spmd

F32 = mybir.dt.float32
BF16 = mybir.dt.bfloat16
FP8 = mybir.dt.float8e4

USE_FP8 = bool(int(os.environ.get("BASS_KERNEL_FP8", "1")))

B_FULL = 32
N_CORES = 8
B_CORE = B_FULL // N_CORES  # 4 images per core
C = 128      # in channels
O = 128      # out channels
H = W = 56
HH = H // 2
HP = 58                      # padded rows
WP = 64 if USE_FP8 else 58   # padded row stride (64 -> vert tap pair = 64B)
IMG = HP * WP                # one padded image copy (3712 fp8), 16-aligned
KS = 3
NTAPS = KS * KS
IKK = C * NTAPS  # 1152
ROWS_PER_TILE = 8           # output rows per PSUM tile -> N = 8*56 = 448
N_TILE = ROWS_PER_TILE * W  # 448 fp32 <= 512 (one PSUM bank)
N_ROW_TILES = H // ROWS_PER_TILE  # 7
ADT = FP8 if USE_FP8 else BF16
SIGN_CHUNK = 14             # x rows per dual-sign chunk (4 chunks)

# lhsT layout inside the packed [C, 1152] weight vector:
#   horiz pairs: ky*256 + j*128 + o   (j = kx in {0,1})          (768)
#   vert pair  : 768 + j*128 + o      (j = ky in {0,1}, kx = 2)  (256)
#   single     : 1024 + o             (ky = 2, kx = 2)           (128)
# Weight-synthesis chunk tiles: c0/c1/c2 = horiz ky row (256 each),
# c3 = vert pair + single (384).  DMA halves split at column 512.
CH_SZ = [256, 256, 256, 384]
ZH = 512  # z/m half split (covers c0+c1; second half covers c2+c3)


def pack_zm(w):
    """(O, C, 3, 3) f32 -> (C, 1152) in lhsT tap order."""
    hp = np.transpose(w[:, :, :, 0:2], (1, 2, 3, 0))      # (C, ky, kx2, O)
    vp = np.transpose(w[:, :, 0:2, 2], (1, 2, 0))         # (C, ky2, O)
    sg = np.transpose(w[:, :, 2, 2], (1, 0))              # (C, O)
    return np.concatenate(
        [hp.reshape(C, -1), vp.reshape(C, -1), sg.reshape(C, -1)], axis=1
    )


def build_program(rv: np.ndarray, n_img: int = B_CORE):
    """Build the per-core Bass program. rv values are baked as immediates."""
    nc = bacc.Bacc(
        "TRN2",
        target_bir_lowering=False,
        debug=False,
        num_devices=N_CORES,
    )

    x_t = nc.dram_tensor("x", (n_img, C, H, W), F32, kind="ExternalInput").ap()
    a_t = nc.dram_tensor("Alpha", (O, 1, 1), F32, kind="ExternalInput").ap()
    # ZM[k] for k<5 is Z_k, ZM[5] is M, all in lhsT tap order.
    zm_t = nc.dram_tensor("ZM", (6, C, IKK), F32, kind="ExternalInput").ap()
    out_t = nc.dram_tensor("out", (n_img, O, H, W), F32, kind="ExternalOutput").ap()

    rv = np.asarray(rv, dtype=np.float32).reshape(-1)
    assert rv.shape[0] == 5

    with tile.TileContext(nc) as tc:
        with (
            tc.tile_pool(name="const", bufs=1) as const_pool,
            tc.tile_pool(name="wsyn", bufs=1) as wsyn_pool,
            tc.tile_pool(name="imgs", bufs=1) as img_pool,
            tc.tile_pool(name="xstage", bufs=8) as x_pool,
            tc.tile_pool(name="evac", bufs=8) as ev_pool,
            tc.tile_pool(name="cpsum", bufs=7, space="PSUM") as cpsum_pool,
        ):
            # --- wire order: alpha, x0h0, M, Z0..Z4 (halved to amortize the
            # ~2.5us completion-receipt lag), x0h1, x1..x3 halves.
            # Halves/chunks are separate tiles so dependency tracking stays
            # at half granularity.
            alpha_sb = const_pool.tile([O, 1], F32)
            nc.sync.dma_start(alpha_sb, a_t.rearrange("o a b -> o (a b)"))

            x_tiles = [None] * n_img  # per image: (half0, half1) tiles

            def alloc_x(img):
                x_tiles[img] = tuple(
                    x_pool.tile([C, HH * W], F32, name=f"x{img}h{hb}", tag="xin")
                    for hb in range(2)
                )

            def load_x_half(img, hb):
                nc.sync.dma_start(
                    x_tiles[img][hb].rearrange("c (h w) -> c h w", w=W),
                    x_t[img, :, hb * HH : (hb + 1) * HH, :],
                )

            def load_halved(dsts, src):
                nc.sync.dma_start(dsts[0], src[:, 0:ZH])
                nc.sync.dma_start(dsts[1], src[:, ZH:IKK])

            def wtiles(name):
                return (
                    wsyn_pool.tile([C, ZH], F32, name=f"{name}h0"),
                    wsyn_pool.tile([C, IKK - ZH], F32, name=f"{name}h1"),
                )

            alloc_x(0)
            load_x_half(0, 0)
            load_x_half(0, 1)
            m_h = wtiles("m")
            load_halved(m_h, zm_t[5])
            z_hs = []
            for k in range(5):
                z_h = wtiles(f"z{k}")
                load_halved(z_h, zm_t[k])
                z_hs.append(z_h)
            for img in range(1, n_img):
                alloc_x(img)
                load_x_half(img, 0)
                load_x_half(img, 1)

            # --- per-image padded sign(x) buffers: copy A (normal) and
            # copy B = A shifted one pixel left, both written by one ACT
            # sign per half-image via a broadcast (stride-0) input dim.
            # Borders zeroed once.
            padded = []
            for img in range(n_img):
                pd = img_pool.tile(
                    [C, 2 * IMG], ADT, name=f"pad{img}", tag=f"pad{img}"
                )
                pdA = pd[:, 0:IMG].rearrange("p (h w) -> p h w", w=WP)
                pdB = pd[:, IMG : 2 * IMG].rearrange("p (h w) -> p h w", w=WP)
                nc.gpsimd.memset(pdA[:, 0, 0:HP], 0.0)
                nc.gpsimd.memset(pdA[:, HP - 1, 0:HP], 0.0)
                nc.gpsimd.memset(pdA[:, 1 : HP - 1, 0:1], 0.0)
                nc.gpsimd.memset(pdA[:, 1 : HP - 1, HP - 1 : HP], 0.0)
                nc.gpsimd.memset(pdB[:, 0, 0:H], 0.0)
                nc.gpsimd.memset(pdB[:, HP - 1, 0:H], 0.0)
                padded.append((pd, pdA, pdB))

            def sign_half(img, hb):
                """sign(x half) -> A interior at (1+r0, 1) and B interior at
                (1+r0, 0) in one ACT op via a broadcast input dim."""
                pd, pdA, pdB = padded[img]
                r0 = hb * HH
                src = x_tiles[img][hb].rearrange("c (h w) -> c h w", w=W)
                src4 = bass.AP(
                    src.tensor,
                    src.offset,
                    [list(src.ap[0]), [0, 2]] + [list(p) for p in src.ap[1:]],
                )
                dstA = pdA[:, 1 + r0 : 1 + r0 + HH, 1 : 1 + W]
                dst4 = bass.AP(
                    dstA.tensor,
                    dstA.offset,
                    [list(dstA.ap[0]), [IMG - 1, 2]]
                    + [list(p) for p in dstA.ap[1:]],
                )
                nc.scalar.sign(dst4, src4)

            sign_half(0, 0)
            sign_half(0, 1)

            # --- weight synthesis: s = M + sum_k rv_k Z_k, already in lhsT
            # layout.  k outer so each round only needs Z_k; chunks c0..c2
            # on DVE, c3 on GpSimd; signs emitted in tile order so the conv
            # (which consumes c0 first) can start on partial weights.
            s_c = [
                wsyn_pool.tile([C, CH_SZ[c]], F32, name=f"s{c}") for c in range(4)
            ]
            bw_c = [
                wsyn_pool.tile([C, CH_SZ[c]], ADT, name=f"bw{c}") for c in range(4)
            ]

            def wslice(halves, c):
                if c == 0:
                    return halves[0][:, 0:256]
                if c == 1:
                    return halves[0][:, 256:512]
                if c == 2:
                    return halves[1][:, 0:256]
                return halves[1][:, 256:640]

            for k in range(5):
                for c in range(4):
                    nc.vector.scalar_tensor_tensor(
                        out=s_c[c],
                        in0=wslice(z_hs[k], c),
                        scalar=float(rv[k]),
                        in1=wslice(m_h, c) if k == 0 else s_c[c],
                        op0=mybir.AluOpType.mult,
                        op1=mybir.AluOpType.add,
                    )
                    if k == 4:
                        nc.scalar.sign(bw_c[c], s_c[c])

            def pair_ap(ct, off):
                return bw_c[ct][:, off : off + 2 * O].rearrange(
                    "c (j o) -> c j o", o=O
                )

            # --- main conv loop; the next image's sign halves are emitted
            # ahead (h0 at loop start, h1 after tile 2) so its pad buffer
            # completes before its own tiles need it.
            for img in range(n_img):
                if img + 1 < n_img:
                    sign_half(img + 1, 0)
                _, pdA, pdB = padded[img]

                for nt in range(N_ROW_TILES):
                    if nt == 3 and img + 1 < n_img:
                        sign_half(img + 1, 1)
                    y0 = nt * ROWS_PER_TILE
                    cv = cpsum_pool.tile([O, N_TILE], F32, tag="cv")
                    if USE_FP8:
                        # horiz pairs (ky, kx0|kx1): step A->B (IMG bytes);
                        # needs bw chunk ky only -> earliest weights first
                        for ky in range(KS):
                            winh = pdA[:, y0 + ky : y0 + ky + ROWS_PER_TILE, 0:W]
                            aph = bass.AP(
                                winh.tensor,
                                winh.offset,
                                [list(winh.ap[0]), [IMG, 2]]
                                + [list(p) for p in winh.ap[1:]],
                            )
                            nc.tensor.matmul(
                                cv,
                                pair_ap(ky, 0),
                                aph,
                                start=(ky == 0),
                                stop=False,
                                perf_mode=mybir.MatmulPerfMode.DoubleRow,
                            )
                        # vert pair (ky0|ky1, kx=2): step WP inside A
                        winv = pdA[:, y0 : y0 + ROWS_PER_TILE, 2 : 2 + W]
                        apv = bass.AP(
                            winv.tensor,
                            winv.offset,
                            [list(winv.ap[0]), [WP, 2]]
                            + [list(p) for p in winv.ap[1:]],
                        )
                        nc.tensor.matmul(
                            cv,
                            pair_ap(3, 0),
                            apv,
                            start=False,
                            stop=False,
                            perf_mode=mybir.MatmulPerfMode.DoubleRow,
                        )
                        # single (ky2, kx2) from A
                        nc.tensor.matmul(
                            cv,
                            bw_c[3][:, 256 : 256 + O],
                            pdA[:, y0 + 2 : y0 + 2 + ROWS_PER_TILE, 2 : 2 + W],
                            start=False,
                            stop=True,
                        )
                    else:
                        t = 0
                        for ky in range(KS):
                            for kx in range(KS):
                                win = pdA[
                                    :,
                                    y0 + ky : y0 + ky + ROWS_PER_TILE,
                                    kx : kx + W,
                                ]
                                if kx < 2:
                                    lhsT = bw_c[ky][:, kx * O : (kx + 1) * O]
                                elif ky < 2:
                                    lhsT = bw_c[3][:, ky * O : (ky + 1) * O]
                                else:
                                    lhsT = bw_c[3][:, 256 : 256 + O]
                                nc.tensor.matmul(
                                    cv,
                                    lhsT,
                                    win,
                                    start=(t == 0),
                                    stop=(t == NTAPS - 1),
                                )
                                t += 1
                    ev = ev_pool.tile([O, N_TILE], F32, tag="ev")
                    nc.vector.tensor_scalar_mul(ev, cv, alpha_sb[:, 0:1])
                    # stores all ride the gpsimd SWDGE queue: the scalar
                    # engine is busy signing images and the sync ring is
                    # busy with x loads
                    nc.gpsimd.dma_start(
                        out_t[img, :, y0 : y0 + ROWS_PER_TILE, :],
                        ev.rearrange("o (h w) -> o h w", w=W),
                    )

    nc.compile()
    return nc


def _ensure_ntff_hook():
    """Register the axon NTFF profiling hook if the image's antenv lacks it.

    Only used when BASS_KERNEL_TRACE=1 (dev profiling); best-effort.
    """
    import sys
    import types

    try:
        import antenv

        if hasattr(antenv, "axon_hooks"):
            return
        mod = types.ModuleType("antenv.axon_hooks")
        _hook = [None]
        mod.set_axon_ntff_profile_hook = lambda h: _hook.__setitem__(0, h)
        mod.get_axon_ntff_profile_hook = lambda: _hook[0]
        sys.modules["antenv.axon_hooks"] = mod
        antenv.axon_hooks = mod
        from trn_agent_boot.trn_boot import _ntff_profile_via_ctypes

        mod.set_axon_ntff_profile_hook(
            _ntff_profile_via_ctypes("/opt/axon/libaxon_pjrt.so")
        )
    except Exception as e:  # pragma: no cover - profiling is optional
        print(f"NTFF hook registration failed ({e}); tracing disabled")


def kernel(x, Alpha, M, Z, rv):
    x = np.ascontiguousarray(np.asarray(x, dtype=np.float32))
    Alpha = np.ascontiguousarray(np.asarray(Alpha, dtype=np.float32))
    M = np.asarray(M, dtype=np.float32)
    Z = np.asarray(Z, dtype=np.float32)
    rv = np.asarray(rv, dtype=np.float32)

    trace = bool(int(os.environ.get("BASS_KERNEL_TRACE", "0")))
    if trace:
        _ensure_ntff_hook()

    nc = build_program(rv)

    zm = np.stack(
        [pack_zm(Z.reshape(5, O, C, KS, KS)[k]) for k in range(5)]
        + [pack_zm(M.reshape(O, C, KS, KS))]
    ).astype(np.float32)
    zm = np.ascontiguousarray(zm)

    in_maps = []
    for c in range(N_CORES):
        in_maps.append(
            {
                "x": np.ascontiguousarray(x[c * B_CORE : (c + 1) * B_CORE]),
                "Alpha": Alpha,
                "ZM": zm,
            }
        )

    res = run_bass_kernel_spmd(
        nc,
        in_maps,
        core_ids=list(range(N_CORES)),
        trace=trace,
    )
    out = np.concatenate([res.results[c]["out"] for c in range(N_CORES)], axis=0)
    if trace:
        kernel.last_results = res
    return out


# revision 23
# speedup vs baseline: 1.2494x; 1.0132x over previous
"""Trainium2 Bass kernel for BinarizeConv2dSDP.

Math (reference):
    s   = M + rv @ Z          (the rsqrt normalization is sign-preserving:
                               w = (m + rv@z) * rsqrt(...) with rsqrt > 0,
                               so sign(w) == sign(s))
    bw  = sign(s)             (O, I, 3, 3)
    ba  = sign(x)             (B, C, H, W)
    out = conv2d(ba, bw, stride 1, pad 1) * Alpha

Strategy:
    - Data-parallel over batch: 8 cores x 4 images each.
    - M/Z are passed host-pretransposed into the matmul lhsT layout so
      weight synthesis is pure elementwise: 5 fused scalar_tensor_tensor
      rounds + sign, chunked into 4 tap-group tiles (3 on DVE, 1 on
      GpSimd) that pipeline behind the halved per-Z DMA arrivals.
    - Each image is signed into TWO fp8 padded buffers: A (normal) and
      B (shifted left one pixel), written by one ACT op per row-chunk via
      a broadcast (stride-0) input dim.  A vertical tap pair steps 64B
      inside A; a horizontal tap pair steps A->B (3712B).  Both are legal
      DoubleRow strides, so each conv row-tile is 4 DoubleRow matmuls
      (K=256) + 1 normal fp8 matmul instead of 9 taps:
        horiz pairs (ky, kx0|kx1) x3  +  vert pair (ky0|ky1, kx2)  +
        single (ky2, kx2)
      +-1 is exact in fp8 and PSUM accumulates f32, so results are exact.
    - Wire order (one HWDGE queue): alpha, x0h0, M, Z0..Z4 (halved),
      x0h1, x1..x3 (halved).  Output stores ride the gpsimd SWDGE queue.
    - Alpha applied during PSUM->SBUF evacuation on DVE; f32 out.
"""

import os
import numpy as np

import concourse.bass as bass
import concourse.tile as tile
from concourse import bacc, mybir
from concourse.bass_utils import run_bass_kernel_spmd

F32 = mybir.dt.float32
BF16 = mybir.dt.bfloat16
FP8 = mybir.dt.float8e4

USE_FP8 = bool(int(os.environ.get("BASS_KERNEL_FP8", "1")))

B_FULL = 32
N_CORES = 8
B_CORE = B_FULL // N_CORES  # 4 images per core
C = 128      # in channels
O = 128      # out channels
H = W = 56
HH = H // 2
HP = 58                      # padded rows
WP = 64 if USE_FP8 else 58   # padded row stride (64 -> vert tap pair = 64B)
IMG = HP * WP                # one padded image copy (3712 fp8), 16-aligned
KS = 3
NTAPS = KS * KS
IKK = C * NTAPS  # 1152
ROWS_PER_TILE = 8           # output rows per PSUM tile -> N = 8*56 = 448
N_TILE = ROWS_PER_TILE * W  # 448 fp32 <= 512 (one PSUM bank)
N_ROW_TILES = H // ROWS_PER_TILE  # 7
ADT = FP8 if USE_FP8 else BF16
SIGN_CHUNK = 14             # x rows per dual-sign chunk (4 chunks)

# lhsT layout inside the packed [C, 1152] weight vector:
#   horiz pairs: ky*256 + j*128 + o   (j = kx in {0,1})          (768)
#   vert pair  : 768 + j*128 + o      (j = ky in {0,1}, kx = 2)  (256)
#   single     : 1024 + o             (ky = 2, kx = 2)           (128)
# Weight-synthesis chunk tiles: c0/c1/c2 = horiz ky row (256 each),
# c3 = vert pair + single (384).  DMA halves split at column 512.
CH_SZ = [256, 256, 256, 384]
ZH = 512  # z/m half split (covers c0+c1; second half covers c2+c3)


def pack_zm(w):
    """(O, C, 3, 3) f32 -> (C, 1152) in lhsT tap order."""
    hp = np.transpose(w[:, :, :, 0:2], (1, 2, 3, 0))      # (C, ky, kx2, O)
    vp = np.transpose(w[:, :, 0:2, 2], (1, 2, 0))         # (C, ky2, O)
    sg = np.transpose(w[:, :, 2, 2], (1, 0))              # (C, O)
    return np.concatenate(
        [hp.reshape(C, -1), vp.reshape(C, -1), sg.reshape(C, -1)], axis=1
    )


def build_program(rv: np.ndarray, n_img: int = B_CORE):
    """Build the per-core Bass program. rv values are baked as immediates."""
    nc = bacc.Bacc(
        "TRN2",
        target_bir_lowering=False,
        debug=False,
        num_devices=N_CORES,
    )

    x_t = nc.dram_tensor("x", (n_img, C, H, W), F32, kind="ExternalInput").ap()
    a_t = nc.dram_tensor("Alpha", (O, 1, 1), F32, kind="ExternalInput").ap()
    # ZM[k] for k<5 is Z_k, ZM[5] is M, all in lhsT tap order.
    zm_t = nc.dram_tensor("ZM", (6, C, IKK), F32, kind="ExternalInput").ap()
    out_t = nc.dram_tensor("out", (n_img, O, H, W), F32, kind="ExternalOutput").ap()

    rv = np.asarray(rv, dtype=np.float32).reshape(-1)
    assert rv.shape[0] == 5

    with tile.TileContext(nc) as tc:
        with (
            tc.tile_pool(name="const", bufs=1) as const_pool,
            tc.tile_pool(name="wsyn", bufs=1) as wsyn_pool,
            tc.tile_pool(name="imgs", bufs=1) as img_pool,
            tc.tile_pool(name="xstage", bufs=8) as x_pool,
            tc.tile_pool(name="evac", bufs=8) as ev_pool,
            tc.tile_pool(name="cpsum", bufs=8, space="PSUM") as cpsum_pool,
        ):
            # --- wire order: alpha, x0h0, M, Z0..Z4 (halved to amortize the
            # ~2.5us completion-receipt lag), x0h1, x1..x3 halves.
            # Halves/chunks are separate tiles so dependency tracking stays
            # at half granularity.
            alpha_sb = const_pool.tile([O, 1], F32)
            nc.sync.dma_start(alpha_sb, a_t.rearrange("o a b -> o (a b)"))

            x_tiles = [None] * n_img  # per image: (half0, half1) tiles

            def alloc_x(img):
                x_tiles[img] = tuple(
                    x_pool.tile([C, HH * W], F32, name=f"x{img}h{hb}", tag="xin")
                    for hb in range(2)
                )

            def load_x_half(img, hb):
                nc.sync.dma_start(
                    x_tiles[img][hb].rearrange("c (h w) -> c h w", w=W),
                    x_t[img, :, hb * HH : (hb + 1) * HH, :],
                )

            def load_halved(dsts, src):
                nc.sync.dma_start(dsts[0], src[:, 0:ZH])
                nc.sync.dma_start(dsts[1], src[:, ZH:IKK])

            def wtiles(name):
                return (
                    wsyn_pool.tile([C, ZH], F32, name=f"{name}h0"),
                    wsyn_pool.tile([C, IKK - ZH], F32, name=f"{name}h1"),
                )

            alloc_x(0)
            load_x_half(0, 0)
            load_x_half(0, 1)
            m_h = wtiles("m")
            load_halved(m_h, zm_t[5])
            z_hs = []
            for k in range(5):
                z_h = wtiles(f"z{k}")
                load_halved(z_h, zm_t[k])
                z_hs.append(z_h)
            for img in range(1, n_img):
                alloc_x(img)
                load_x_half(img, 0)
                load_x_half(img, 1)

            # --- per-image padded sign(x) buffers: copy A (normal) and
            # copy B = A shifted one pixel left, both written by one ACT
            # sign per half-image via a broadcast (stride-0) input dim.
            # Borders zeroed once.
            padded = []
            for img in range(n_img):
                pd = img_pool.tile(
                    [C, 2 * IMG], ADT, name=f"pad{img}", tag=f"pad{img}"
                )
                pdA = pd[:, 0:IMG].rearrange("p (h w) -> p h w", w=WP)
                pdB = pd[:, IMG : 2 * IMG].rearrange("p (h w) -> p h w", w=WP)
                nc.gpsimd.memset(pdA[:, 0, 0:HP], 0.0)
                nc.gpsimd.memset(pdA[:, HP - 1, 0:HP], 0.0)
                nc.gpsimd.memset(pdA[:, 1 : HP - 1, 0:1], 0.0)
                nc.gpsimd.memset(pdA[:, 1 : HP - 1, HP - 1 : HP], 0.0)
                nc.gpsimd.memset(pdB[:, 0, 0:H], 0.0)
                nc.gpsimd.memset(pdB[:, HP - 1, 0:H], 0.0)
                padded.append((pd, pdA, pdB))

            def sign_half(img, hb):
                """sign(x half) -> A interior at (1+r0, 1) and B interior at
                (1+r0, 0) in one ACT op via a broadcast input dim."""
                pd, pdA, pdB = padded[img]
                r0 = hb * HH
                src = x_tiles[img][hb].rearrange("c (h w) -> c h w", w=W)
                src4 = bass.AP(
                    src.tensor,
                    src.offset,
                    [list(src.ap[0]), [0, 2]] + [list(p) for p in src.ap[1:]],
                )
                dstA = pdA[:, 1 + r0 : 1 + r0 + HH, 1 : 1 + W]
                dst4 = bass.AP(
                    dstA.tensor,
                    dstA.offset,
                    [list(dstA.ap[0]), [IMG - 1, 2]]
                    + [list(p) for p in dstA.ap[1:]],
                )
                nc.scalar.sign(dst4, src4)

            sign_half(0, 0)
            sign_half(0, 1)

            # --- weight synthesis: s = M + sum_k rv_k Z_k, already in lhsT
            # layout.  k outer so each round only needs Z_k; chunks c0..c2
            # on DVE, c3 on GpSimd; signs emitted in tile order so the conv
            # (which consumes c0 first) can start on partial weights.
            s_c = [
                wsyn_pool.tile([C, CH_SZ[c]], F32, name=f"s{c}") for c in range(4)
            ]
            bw_c = [
                wsyn_pool.tile([C, CH_SZ[c]], ADT, name=f"bw{c}") for c in range(4)
            ]

            def wslice(halves, c):
                if c == 0:
                    return halves[0][:, 0:256]
                if c == 1:
                    return halves[0][:, 256:512]
                if c == 2:
                    return halves[1][:, 0:256]
                return halves[1][:, 256:640]

            for k in range(5):
                for c in range(4):
                    nc.vector.scalar_tensor_tensor(
                        out=s_c[c],
                        in0=wslice(z_hs[k], c),
                        scalar=float(rv[k]),
                        in1=wslice(m_h, c) if k == 0 else s_c[c],
                        op0=mybir.AluOpType.mult,
                        op1=mybir.AluOpType.add,
                    )
                    if k == 4:
                        nc.scalar.sign(bw_c[c], s_c[c])

            def pair_ap(ct, off):
                return bw_c[ct][:, off : off + 2 * O].rearrange(
                    "c (j o) -> c j o", o=O
                )

            # --- main conv loop; the next image's sign halves are emitted
            # ahead (h0 at loop start, h1 after tile 2) so its pad buffer
            # completes before its own tiles need it.
            for img in range(n_img):
                if img + 1 < n_img:
                    sign_half(img + 1, 0)
                _, pdA, pdB = padded[img]

                for nt in range(N_ROW_TILES):
                    if nt == 3 and img + 1 < n_img:
                        sign_half(img + 1, 1)
                    y0 = nt * ROWS_PER_TILE
                    cv = cpsum_pool.tile([O, N_TILE], F32, tag="cv")
                    if USE_FP8:
                        # horiz pairs (ky, kx0|kx1): step A->B (IMG bytes);
                        # needs bw chunk ky only -> earliest weights first
                        for ky in range(KS):
                            winh = pdA[:, y0 + ky : y0 + ky + ROWS_PER_TILE, 0:W]
                            aph = bass.AP(
                                winh.tensor,
                                winh.offset,
                                [list(winh.ap[0]), [IMG, 2]]
                                + [list(p) for p in winh.ap[1:]],
                            )
                            nc.tensor.matmul(
                                cv,
                                pair_ap(ky, 0),
                                aph,
                                start=(ky == 0),
                                stop=False,
                                perf_mode=mybir.MatmulPerfMode.DoubleRow,
                            )
                        # vert pair (ky0|ky1, kx=2): step WP inside A
                        winv = pdA[:, y0 : y0 + ROWS_PER_TILE, 2 : 2 + W]
                        apv = bass.AP(
                            winv.tensor,
                            winv.offset,
                            [list(winv.ap[0]), [WP, 2]]
                            + [list(p) for p in winv.ap[1:]],
                        )
                        nc.tensor.matmul(
                            cv,
                            pair_ap(3, 0),
                            apv,
                            start=False,
                            stop=False,
                            perf_mode=mybir.MatmulPerfMode.DoubleRow,
                        )
                        # single (ky2, kx2) from A
                        nc.tensor.matmul(
                            cv,
                            bw_c[3][:, 256 : 256 + O],
                            pdA[:, y0 + 2 : y0 + 2 + ROWS_PER_TILE, 2 : 2 + W],
                            start=False,
                            stop=True,
                        )
                    else:
                        t = 0
                        for ky in range(KS):
                            for kx in range(KS):
                                win = pdA[
                                    :,
                                    y0 + ky : y0 + ky + ROWS_PER_TILE,
                                    kx : kx + W,
                                ]
                                if kx < 2:
                                    lhsT = bw_c[ky][:, kx * O : (kx + 1) * O]
                                elif ky < 2:
                                    lhsT = bw_c[3][:, ky * O : (ky + 1) * O]
                                else:
                                    lhsT = bw_c[3][:, 256 : 256 + O]
                                nc.tensor.matmul(
                                    cv,
                                    lhsT,
                                    win,
                                    start=(t == 0),
                                    stop=(t == NTAPS - 1),
                                )
                                t += 1
                    ev = ev_pool.tile([O, N_TILE], F32, tag="ev")
                    nc.vector.tensor_scalar_mul(ev, cv, alpha_sb[:, 0:1])
                    # stores ride the gpsimd SWDGE queue (the scalar engine
                    # is busy signing and the sync ring carries x loads);
                    # the last image's stores alternate with the scalar
                    # ring (free by then) so the tail drains in parallel
                    if img == n_img - 1 and nt % 2 == 1:
                        st_eng = nc.scalar
                    else:
                        st_eng = nc.gpsimd
                    st_eng.dma_start(
                        out_t[img, :, y0 : y0 + ROWS_PER_TILE, :],
                        ev.rearrange("o (h w) -> o h w", w=W),
                    )

    nc.compile()
    return nc


def _ensure_ntff_hook():
    """Register the axon NTFF profiling hook if the image's antenv lacks it.

    Only used when BASS_KERNEL_TRACE=1 (dev profiling); best-effort.
    """
    import sys
    import types

    try:
        import antenv

        if hasattr(antenv, "axon_hooks"):
            return
        mod = types.ModuleType("antenv.axon_hooks")
        _hook = [None]
        mod.set_axon_ntff_profile_hook = lambda h: _hook.__setitem__(0, h)
        mod.get_axon_ntff_profile_hook = lambda: _hook[0]
        sys.modules["antenv.axon_hooks"] = mod
        antenv.axon_hooks = mod
        from trn_agent_boot.trn_boot import _ntff_profile_via_ctypes

        mod.set_axon_ntff_profile_hook(
            _ntff_profile_via_ctypes("/opt/axon/libaxon_pjrt.so")
        )
    except Exception as e:  # pragma: no cover - profiling is optional
        print(f"NTFF hook registration failed ({e}); tracing disabled")


def kernel(x, Alpha, M, Z, rv):
    x = np.ascontiguousarray(np.asarray(x, dtype=np.float32))
    Alpha = np.ascontiguousarray(np.asarray(Alpha, dtype=np.float32))
    M = np.asarray(M, dtype=np.float32)
    Z = np.asarray(Z, dtype=np.float32)
    rv = np.asarray(rv, dtype=np.float32)

    trace = bool(int(os.environ.get("BASS_KERNEL_TRACE", "0")))
    if trace:
        _ensure_ntff_hook()

    nc = build_program(rv)

    zm = np.stack(
        [pack_zm(Z.reshape(5, O, C, KS, KS)[k]) for k in range(5)]
        + [pack_zm(M.reshape(O, C, KS, KS))]
    ).astype(np.float32)
    zm = np.ascontiguousarray(zm)

    in_maps = []
    for c in range(N_CORES):
        in_maps.append(
            {
                "x": np.ascontiguousarray(x[c * B_CORE : (c + 1) * B_CORE]),
                "Alpha": Alpha,
                "ZM": zm,
            }
        )

    res = run_bass_kernel_spmd(
        nc,
        in_maps,
        core_ids=list(range(N_CORES)),
        trace=trace,
    )
    out = np.concatenate([res.results[c]["out"] for c in range(N_CORES)], axis=0)
    if trace:
        kernel.last_results = res
    return out
